# revision 14
# baseline (speedup 1.0000x reference)
# Trainium2 Bass kernel for nn_CapsuleLayer (dynamic-routing capsule layer).
#
# Math reformulation (exact, not approximate):
#   einsum('bni,njkl->bnjl', x, W) contracts i and k independently, so
#     predictions[b,n,j,l] = xs[b,n] * Ws[n,j,l]
#   with xs = x.sum(-1) [B,N], Ws = W.sum(2) [N,J,L].
#   Routing logits b start at 0 and update b += xs * (Ws . v), which is linear
#   in v, so the [B,N,J] logit state never needs to be materialized:
#     c_i = softmax_j( xs[b,n] * (Ws[n,j,:] . Vacc[b,j,:]) ),  Vacc = sum of past v's
#   Each iteration:
#     u[b,n,j] = xs[b,n] * c[b,n,j]
#     s[b,j,l] = sum_n u[b,n,j] * Ws[n,j,l]      (PE cross-product + diag extract)
#     v = squash(s)
#
# Sharding (v2): batch B=64 split over 8 cores (8 samples each) for the
# routing; W sharded over n (288 rows per core).  Each core reads only its
# 4.7MB W slice, k-reduces it to Ws_chunk [288, 512], and ONE bf16 AllGather
# replicates the full Ws [2304, 512] to every core (3.5x less HBM time than
# the baseline's replicated 37.7MB W read per core).

import numpy as np

import concourse.bass as bass
import concourse.mybir as mybir
import concourse.tile as tile
from concourse import bacc
from concourse import bass_utils
from concourse.bass import ts
from concourse.masks import make_identity

B, N, DI, J, L = 64, 2304, 8, 32, 16
NCORES = 8
BC = B // NCORES          # 8 samples per core
NSH = N // NCORES         # 288 n-rows per core (W shard)
P = 128
NT = N // P               # 18 n-tiles
JL = J * L                # 512
BJ = BC * J               # 256
NG = JL // P              # 4 partition groups of the (j,l) axis
EPS = 1e-7
GROUPS = [list(range(NCORES))]

F32 = mybir.dt.float32
F32R = mybir.dt.float32r
BF16 = mybir.dt.bfloat16
AX = mybir.AxisListType
AF = mybir.ActivationFunctionType
ALU = mybir.AluOpType


def r(ap):
    """bitcast to float32r for full-rate PE matmuls (moving dim >= 256)."""
    return ap.bitcast(F32R)


def _emit(ctx, tc, x_ap, w_ap, v_ap, stage=7):
    nc = tc.nc

    # ---------------- pools ----------------
    const = ctx.enter_context(tc.tile_pool(name="const", bufs=1))
    wstream = ctx.enter_context(tc.tile_pool(name="wstream", bufs=3))
    xio = ctx.enter_context(tc.tile_pool(name="xio", bufs=2))
    work = ctx.enter_context(tc.tile_pool(name="work", bufs=4))
    small = ctx.enter_context(tc.tile_pool(name="small", bufs=4))
    tailp = ctx.enter_context(tc.tile_pool(name="tailp", bufs=2))
    dram = ctx.enter_context(
        tc.tile_pool(name="dram", bufs=max(2, stage % 100 if stage >= 200 else (stage - 100 if stage >= 100 else 2)), space="DRAM")
    )
    ps_xs = ctx.enter_context(tc.tile_pool(name="ps_xs", bufs=1, space="PSUM"))
    ps_wT = ctx.enter_context(tc.tile_pool(name="ps_wT", bufs=1, space="PSUM"))
    ps_A = ctx.enter_context(tc.tile_pool(name="ps_A", bufs=3, space="PSUM"))
    ps_S = ctx.enter_context(tc.tile_pool(name="ps_S", bufs=1, space="PSUM"))
    ps_vT = ctx.enter_context(tc.tile_pool(name="ps_vT", bufs=1, space="PSUM"))

    # ---------------- constants ----------------
    id128 = const.tile([P, P], F32, name="id128")
    make_identity(nc, id128)
    id128r = const.tile([P, P], F32R, name="id128r")
    nc.vector.tensor_copy(out=id128r, in_=id128)
    id8 = const.tile([BC, BC], F32, name="id8")
    make_identity(nc, id8)

    # Mdiag[p, 0, k] = 1.0 iff k == p % 32   (diagonal-block extraction mask)
    Mdiag = const.tile([P, 1, 1, J], F32, name="Mdiag")
    nc.gpsimd.memset(Mdiag, 0.0)
    for q in range(P // J):
        nc.gpsimd.affine_select(
            out=Mdiag[ts(q, J), 0, 0, :],
            in_=Mdiag[ts(q, J), 0, 0, :],
            compare_op=ALU.not_equal,
            fill=1.0,
            base=0,
            pattern=[[-1, J]],
            channel_multiplier=1,
        )

    # maskbd[p, g, j] = 1.0 iff j == g*8 + p//16  (block-diag v builder),
    # i.e. iff (p + 128g - 16j) in [0, 15]. Built arithmetically because
    # engine ops require 32-aligned partition starts.
    jiota = const.tile([P, 1, J], F32, name="jiota")
    nc.gpsimd.iota(
        out=jiota,
        pattern=[[0, 1], [1, J]],
        base=0,
        channel_multiplier=0,
        allow_small_or_imprecise_dtypes=True,
    )
    piota = const.tile([P, 1, 1], F32, name="piota")
    nc.gpsimd.iota(
        out=piota,
        pattern=[[0, 1]],
        base=0,
        channel_multiplier=1,
        allow_small_or_imprecise_dtypes=True,
    )
    maskbd = const.tile([P, NG, 1, J], F32, name="maskbd")
    pg = const.tile([P, 1, 1], F32, name="pg")
    q_t = const.tile([P, 1, J], F32, name="q_t")
    qa = const.tile([P, 1, J], F32, name="qa")
    qb = const.tile([P, 1, J], F32, name="qb")
    for g in range(NG):
        nc.vector.tensor_scalar_add(out=pg, in0=piota, scalar1=float(P * g))
        nc.vector.tensor_scalar(
            out=q_t, in0=jiota, scalar1=-float(L), scalar2=pg,
            op0=ALU.mult, op1=ALU.add,
        )
        nc.vector.tensor_scalar(
            out=qa, in0=q_t, scalar1=0.0, scalar2=None, op0=ALU.is_ge
        )
        nc.vector.tensor_scalar(
            out=qb, in0=q_t, scalar1=float(L - 1), scalar2=None, op0=ALU.is_le
        )
        nc.vector.tensor_mul(out=maskbd[:, g], in0=qa, in1=qb)

    eps_ap = const.tile([P, 1], F32, name="eps_ap")
    nc.gpsimd.memset(eps_ap, EPS)

    # REP[p, (jm l)] = 1.0 iff l == p  (partition-replication stationary matrix:
    # out = REP.T @ rhs copies rhs's 16 partitions to all 8 16-partition groups)
    REP = const.tile([L, P], F32, name="REP")
    nc.gpsimd.memset(REP, 0.0)
    nc.gpsimd.affine_select(
        out=REP.rearrange("p (jm l) -> p jm l", l=L),
        in_=REP.rearrange("p (jm l) -> p jm l", l=L),
        compare_op=ALU.not_equal,
        fill=1.0,
        base=0,
        pattern=[[0, P // L], [1, L]],
        channel_multiplier=-1,
    )
    REPr = const.tile([L, P], F32R, name="REPr")
    nc.vector.tensor_copy(out=REPr, in_=REP)

    # ---------------- persistent tensors ----------------
    xs = const.tile([P, NT, BC, 1], F32, name="xs")        # xs[n%128, n//128, b]
    Ws = const.tile([P, NT, JL], BF16, name="Ws")           # Ws[n%128, n//128, (j l)]
    WsT = const.tile([P, NG, N], BF16, name="WsT")          # WsT[(j l)%128, (j l)//128, n]
    Vrep = const.tile([P, BJ], F32, name="Vrep")           # Vacc[l, (b j)] replicated x8 over partitions

    # ---------------- x prep: xs[n, b] = sum_i x[b, n, i], transposed ----------------
    def x_prep():
        xs_ps = ps_xs.tile([P, NT, BC, 1], F32, name="xs_ps")
        for t in range(NT):
            x_t = xio.tile([BC, P, DI], F32, name="x_t", tag="x_t")
            nc.scalar.dma_start(out=x_t, in_=x_ap[:, ts(t, P), :])
            xsb_t = xio.tile([BC, P], F32, name="xsb_t", tag="xsb_t")
            nc.vector.reduce_sum(out=xsb_t, in_=x_t, axis=AX.X)
            nc.tensor.transpose(out=xs_ps[:, t, :, 0], in_=xsb_t, identity=id8)
        nc.vector.tensor_copy(out=xs, in_=xs_ps)
        return xs_ps

    # ---------------- W shard phase: read 1/8 of W, k-reduce, bf16 AllGather --
    # W slice tiles: t=0,1 are full 128 rows, t=2 is the 32-row tail.
    WT_ROWS = [P, P, NSH - 2 * P]

    def w_shard_phase():
        ag_in = dram.tile([NSH, JL], BF16, name="ag_in", tag="ag_in")
        for t, rows in enumerate(WT_ROWS):
            w_t = wstream.tile([P, J, DI, L], F32, name="w_t", tag="w_t")
            wv = w_t[0:rows]
            nc.sync.dma_start(out=wv, in_=w_ap[t * P : t * P + rows])
            ws_t = wstream.tile([P, JL], BF16, name="ws_t", tag="ws_t")
            if t == 2:
                # gpsimd in-place add-tree for the small tail tile
                nc.gpsimd.tensor_add(
                    out=wv[:, :, 0 : DI // 2, :],
                    in0=wv[:, :, 0 : DI // 2, :],
                    in1=wv[:, :, DI // 2 : DI, :],
                )
                nc.gpsimd.tensor_add(
                    out=wv[:, :, 0 : DI // 4, :],
                    in0=wv[:, :, 0 : DI // 4, :],
                    in1=wv[:, :, DI // 4 : DI // 2, :],
                )
                nc.gpsimd.tensor_add(
                    out=ws_t[0:rows].rearrange("p (j l) -> p j l", j=J),
                    in0=wv[:, :, 0, :],
                    in1=wv[:, :, 1, :],
                )
            else:
                nc.vector.reduce_sum(
                    out=ws_t[0:rows].rearrange("p (j l) -> p j l", j=J),
                    in_=wv.rearrange("p j k l -> p j l k"),
                    axis=AX.X,
                )
            nc.sync.dma_start(out=ag_in[t * P : t * P + rows], in_=ws_t[0:rows])
        return ag_in

    def w_allgather(ag_in):
        ag_out = dram.tile([N, JL], BF16, name="ag_out", tag="ag_out")
        nc.gpsimd.collective_compute(
            "AllGather",
            ALU.bypass,
            replica_groups=GROUPS,
            ins=[ag_in[:].opt()],
            outs=[ag_out[:].opt()],
        )
        return ag_out

    # ---------------- gathered-Ws load: raw bf16 + xbar DMA transposes -------
    def w_load(ag_out):
        nc.sync.dma_start(out=Ws, in_=ag_out.rearrange("(t p) c -> p t c", p=P))
        for g in range(NG):
            nc.scalar.dma_start(
                out=WsT[:, g, :], in_=ag_out[:, ts(g, P)], transpose=True
            )

    # ---------------- one routing iteration ----------------
    def routing_iter(it, bd, xs_ps):
        """it in {1,2,3}; bd is the block-diag Vacc tensor (None for it==1).
        Returns v_a [P, 2, L] where row p of half h holds v[b, j, :] with
        b = 4*h + p//32, j = p % 32."""
        psS = [
            ps_S.tile([P, JL], F32, name=f"psS{h}_{it}", tag=f"psS{h}")
            for h in range(2)
        ]
        for tp in range(0, NT, 2):
            u2 = work.tile([P, 2, BJ], BF16, name="u2", tag="u")
            u2v = u2.rearrange("p d (b j) -> p d b j", b=BC)
            if it == 1:
                # c is uniform 1/J: u = xs / J, broadcast over j.
                nc.vector.tensor_scalar(
                    out=u2v,
                    in0=xs_ps[:, tp : tp + 2].to_broadcast([P, 2, BC, J]),
                    scalar1=1.0 / J,
                    scalar2=None,
                    op0=ALU.mult,
                )
            else:
                psA2 = ps_A.tile([P, 2, BC, J], F32, name="psA2", tag="psA")
                for dt in range(2):
                    for g in range(NG):
                        nc.tensor.matmul(
                            psA2[:, dt],
                            lhsT=WsT[:, g, ts(tp + dt, P)],
                            rhs=bd[:, g, :, :],
                            start=(g == 0),
                            stop=(g == NG - 1),
                        )
                # logits = xs * A ; c = softmax_j ; u = xs * c
                Lt2 = work.tile([P, 2, BC, J], BF16, name="Lt2", tag="Lt")
                nc.vector.tensor_mul(
                    out=Lt2,
                    in0=psA2,
                    in1=xs[:, tp : tp + 2].to_broadcast([P, 2, BC, J]),
                )
                Et2 = work.tile([P, 2, BC, J], BF16, name="Et2", tag="Et")
                nc.scalar.activation(out=Et2, in_=Lt2, func=AF.Exp)
                St2 = small.tile([P, 2, BC, 1], F32, name="St2", tag="St")
                nc.vector.reduce_sum(out=St2, in_=Et2, axis=AX.X)
                Rt2 = small.tile([P, 2, BC, 1], F32, name="Rt2", tag="Rt")
                nc.vector.reciprocal(out=Rt2, in_=St2)
                xsR2 = small.tile([P, 2, BC, 1], BF16, name="xsR2", tag="xsR")
                nc.vector.tensor_mul(out=xsR2, in0=Rt2, in1=xs[:, tp : tp + 2])
                # u-mult alternates DVE / GPSIMD per pair to balance engines
                eng = nc.vector if (tp // 2) % 2 else nc.gpsimd
                eng.tensor_mul(
                    out=u2v,
                    in0=Et2,
                    in1=xsR2.to_broadcast([P, 2, BC, J]),
                )
            for dt in range(2):
                t = tp + dt
                for h in range(2):
                    nc.tensor.matmul(
                        psS[h],
                        lhsT=u2[:, dt, ts(h, P)],
                        rhs=Ws[:, t, :],
                        start=(t == 0),
                        stop=(t == NT - 1),
                    )

        # ---- diagonal extraction: s[p, h, l] from psS[h][(b j), (j' l)] ----
        s_a = tailp.tile([P, 2, L], F32, name="s_a", tag="s_a")
        for h in range(2):
            dtmp = tailp.tile([P, L, J], F32, name="dtmp", tag="dtmp")
            nc.vector.tensor_mul(
                out=dtmp,
                in0=psS[h].rearrange("p (k l) -> p l k", k=J),
                in1=Mdiag[:, 0].to_broadcast([P, L, J]),
            )
            nc.vector.reduce_sum(out=s_a[:, h, :], in_=dtmp, axis=AX.X)

        # ---- squash: v = s * n/(1+n)/sqrt(n+eps) ----
        nrm = tailp.tile([P, 2, 1], F32, name="nrm", tag="nrm")
        sq = tailp.tile([P, 2, L], F32, name="sq", tag="sq")
        nc.vector.tensor_mul(out=sq, in0=s_a, in1=s_a)
        nc.vector.reduce_sum(out=nrm, in_=sq, axis=AX.X)
        d1 = tailp.tile([P, 2, 1], F32, name="d1", tag="d1")
        nc.vector.tensor_scalar_add(out=d1, in0=nrm, scalar1=1.0)
        sqt = tailp.tile([P, 2, 1], F32, name="sqt", tag="sqt")
        nc.scalar.activation(out=sqt, in_=nrm, func=AF.Sqrt, bias=eps_ap)
        den = tailp.tile([P, 2, 1], F32, name="den", tag="den")
        nc.vector.tensor_mul(out=den, in0=d1, in1=sqt)
        rec = tailp.tile([P, 2, 1], F32, name="rec", tag="rec")
        nc.vector.reciprocal(out=rec, in_=den)
        # v = (s * rec) * nrm, fused per half via dual-scalar tensor_scalar
        v_a = tailp.tile([P, 2, L], F32R, name="v_a", tag="v_a")
        for h in range(2):
            nc.vector.tensor_scalar(
                out=v_a[:, h, :],
                in0=s_a[:, h, :],
                scalar1=rec[:, h, :],
                scalar2=nrm[:, h, :],
                op0=ALU.mult,
                op1=ALU.mult,
            )
        return v_a

    def accumulate_v(v_a, first):
        """Transpose v_a into vT[l, (b j)] (replicated over 8 partition groups)
        and accumulate into Vrep; build block-diag bd for the next iteration."""
        vT = ps_vT.tile([L, BJ], F32R, name="vT", tag="vT")
        for h in range(2):
            nc.tensor.transpose(
                out=vT[:, ts(h, P)], in_=v_a[:, h, :], identity=id128r
            )
        vT_sb = tailp.tile([L, BJ], F32R, name="vT_sb", tag="vT_sb")
        nc.vector.tensor_copy(out=vT_sb, in_=vT)
        vrep_ps = ps_wT.tile([P, BJ], F32, name="vrep_ps", tag="ps_share")
        nc.tensor.matmul(vrep_ps, lhsT=REPr, rhs=vT_sb, start=True, stop=True)
        if first:
            nc.vector.tensor_copy(out=Vrep, in_=vrep_ps)
        else:
            nc.vector.tensor_add(out=Vrep, in0=Vrep, in1=vrep_ps)
        bd = tailp.tile([P, NG, BC, J], BF16, name="bd", tag="bd")
        for g in range(NG):
            nc.gpsimd.tensor_mul(
                out=bd[:, g, :, :],
                in0=Vrep.rearrange("p (b j) -> p b j", b=BC),
                in1=maskbd[:, g].to_broadcast([P, BC, J]),
            )
        return bd

    # ---------------- main schedule ----------------
    v_flat = v_ap.rearrange("b j l -> (b j) l")

    def emit_out(v_x):
        for h in range(2):
            nc.sync.dma_start(out=v_flat[ts(h, P)], in_=v_x[:, h, :].bitcast(F32))

    def emit_stub(src_sb):
        stub = const.tile([P, 2, L], F32R, name="stub")
        nc.vector.tensor_scalar(
            out=stub,
            in0=src_sb.to_broadcast([P, 2, L]),
            scalar1=1.0,
            scalar2=None,
            op0=ALU.mult,
        )
        emit_out(stub)

    def one_pass(upto=7):
        xs_ps = x_prep()
        ag_in = w_shard_phase()
        if upto == 2:
            emit_stub(xs_ps[:, 0, 0:1, :])
            return
        ag_out = w_allgather(ag_in)
        if upto == 3:
            probe = wstream.tile([P, 1, L], BF16, name="probe", tag="probe")
            nc.sync.dma_start(out=probe[:, 0, :], in_=ag_out[0:P, 0:L])
            probef = wstream.tile([P, 1, L], F32, name="probef", tag="probef")
            nc.vector.tensor_copy(out=probef, in_=probe)
            emit_stub(probef[:, 0:1, 0:1])
            return
        w_load(ag_out)
        if upto == 4:
            emit_stub(Ws[:, 0:1, 0:2].bitcast(F32))
            return
        v1 = routing_iter(1, None, xs_ps)
        if upto == 5:
            emit_out(v1)
            return
        bd1 = accumulate_v(v1, first=True)
        v2 = routing_iter(2, bd1, xs_ps)
        if upto == 6:
            emit_out(v2)
            return
        bd2 = accumulate_v(v2, first=False)
        v3 = routing_iter(3, bd2, xs_ps)
        emit_out(v3)

    if stage >= 200:
        upto, reps = divmod(stage, 100)
        for i in range(reps):
            if i:
                tc.strict_bb_all_engine_barrier()
            one_pass(upto)
    elif stage >= 100:
        for i in range(stage - 100):
            if i:
                tc.strict_bb_all_engine_barrier()
            one_pass(7)
    else:
        one_pass(stage)


_nc_cache = {}


def build(stage=7):
    if stage not in _nc_cache:
        from contextlib import ExitStack

        nc = bacc.Bacc(
            "TRN2", target_bir_lowering=False, debug=False, num_devices=NCORES
        )
        x_ap = nc.dram_tensor("x", [BC, N, DI], F32, kind="ExternalInput").ap()
        w_ap = nc.dram_tensor("w", [NSH, J, DI, L], F32, kind="ExternalInput").ap()
        v_ap = nc.dram_tensor("v", [BC, J, L], F32, kind="ExternalOutput").ap()
        with (
            tile.TileContext(nc) as tc,
            ExitStack() as ctx,
            nc.allow_low_precision(
                reason="f32r is a rounded fp32 view required for full-rate PE "
                "matmuls; Ws is exchanged in bf16 (0.4% rounding, well within "
                "the 2e-2 tolerance); accumulation happens in fp32 PSUM"
            ),
        ):
            _emit(ctx, tc, x_ap, w_ap, v_ap, stage=stage)
        nc.compile()
        _nc_cache[stage] = nc
    return _nc_cache[stage]


def make_in_maps(x, W):
    return [
        {
            "x": x[i * BC : (i + 1) * BC],
            "w": W[i * NSH : (i + 1) * NSH],
        }
        for i in range(NCORES)
    ]


def run(x, W, trace=False, trace_kwargs=None):
    x = np.ascontiguousarray(np.asarray(x, dtype=np.float32))
    W = np.ascontiguousarray(np.asarray(W, dtype=np.float32))
    assert x.shape == (B, N, DI) and W.shape == (N, J, DI, L)
    nc = build()
    in_maps = make_in_maps(x, W)
    res = bass_utils.run_bass_kernel_spmd(
        nc,
        in_maps,
        core_ids=list(range(NCORES)),
        trace=trace,
        **(trace_kwargs or {}),
    )
    out = np.concatenate([res.results[i]["v"] for i in range(NCORES)], axis=0)
    return out, res


def kernel(**inputs):
    x = inputs["x"]
    W = inputs["W"]
    out, _ = run(x, W, trace=False)
    return out
